# revision 37
# baseline (speedup 1.0000x reference)
"""Trainium2 Bass kernel for nn_AbilityGammaAttention.

Reference computation (per batch b):
    ws = s_j @ Ws_w.T + Ws_b                      # (P, A)
    uh = exp_tokens @ U_w.T                       # (Q, LE, A)
    e[q,p,t] = v . tanh(uh[q,t,:] + ws[p,:])      # (Q, P, LE)
    e masked by exp_mask (tokens), joint softmax over (Q, LE) per (b, p)
    out[q,p,:] = sum_t a[q,p,t] * exp_tokens[q,t,:], zeroed where req_mask[p]==0

Sharding: data-parallel over B across the 8 NeuronCores (batch b -> core b).

Per-core design:
  - uh is computed transposed (uhT: [A=128 partitions, tokens]) so the per-p
    "+ ws[p,:]" is a per-partition bias fused into the ScalarE tanh (the
    ScalarE tanh over P x tokens x A elements is the bottleneck engine).
  - e for all 32 p is accumulated directly into a PSUM tile [P, tokens-half]
    using a selector weight matrix (column p of slice p = v, rest zeros), so
    per-p PSUM evacuation is avoided; exp_mask lands as an extra accumulated
    (m-1)*1e9 rank-1 matmul, exactly reproducing the reference -1e9 masking.
  - Tokens are processed in two halves with the x-load/transpose/uh prep of
    half 1 overlapping the tanh loop of half 0.
  - Matmuls run in float32r (TF32-like); the tanh->e path runs in bf16
    (tanh output is in [-1,1]).
  - Softmax uses a data-independent shift (bound = sum|v_w|, computed on the
    host at first call) instead of a row max, so exp can never overflow and
    both reduce_max passes disappear; denominators come free via the ScalarE
    Exp accum_out, and the normalization (and req_mask) is folded into the
    per-partition scale of the PSUM->SBUF output copies.
"""

import sys

if "/opt/trn_rl_repo" not in sys.path:
    sys.path.insert(0, "/opt/trn_rl_repo")

import numpy as np

import concourse.bacc as bacc
import concourse.mybir as mybir
from concourse.masks import make_identity
from concourse.tile import TileContext

F32 = mybir.dt.float32
F32R = mybir.dt.float32r
BF16 = mybir.dt.bfloat16
I32 = mybir.dt.int32
AF = mybir.ActivationFunctionType
ALU = mybir.AluOpType

B, Q, LE, D, P, A = 8, 32, 128, 512, 32, 128
N_CORES = 8


def build_kernel(q=Q, bound=12.0):
    """Build the per-core kernel (one batch per core). q must be a multiple of 8.

    `bound` is any value >= max possible |e| = sum(|v_w|); exp is shifted by it
    instead of a computed row max (softmax is shift-invariant, and exp(e-bound)
    can never overflow)."""
    T = q * LE          # tokens per batch
    DC = D // 128       # contraction chunks (4)
    T2 = T // 2         # tokens per half
    NCK2 = T2 // 512    # 512-column (one PSUM bank) chunks per half
    QH = q // 2         # q per half
    assert NCK2 * 512 == T2 and QH % 4 == 0

    nc = bacc.Bacc("TRN2", target_bir_lowering=False, debug=False)

    x_dram = nc.dram_tensor("exp_tokens", [q, LE, D], F32, kind="ExternalInput")
    m_dram = nc.dram_tensor("exp_mask", [q, LE], I32, kind="ExternalInput")
    sj_dram = nc.dram_tensor("s_j", [P, D], F32, kind="ExternalInput")
    rm_dram = nc.dram_tensor("req_mask", [P], I32, kind="ExternalInput")
    wsw_dram = nc.dram_tensor("Ws_w", [A, D], F32, kind="ExternalInput")
    wsb_dram = nc.dram_tensor("Ws_b", [A], F32, kind="ExternalInput")
    uw_dram = nc.dram_tensor("U_w", [A, D], F32, kind="ExternalInput")
    vw_dram = nc.dram_tensor("v_w", [1, A], F32, kind="ExternalInput")
    out_dram = nc.dram_tensor("out", [q, P, D], F32, kind="ExternalOutput")

    with TileContext(nc) as tc:
        with tc.tile_pool(name="live", bufs=1) as L:
            # ---- whole-kernel tensors (base of the SBUF stack) ----------
            ident = L.tile([128, 128], F32)
            identr = L.tile([128, 128], F32R)
            x_all = L.tile([128, q * D], F32R)     # x[i] natural at cols i*D
            uhT = L.tile([A, T], F32)              # U_w @ x.T
            wsT = L.tile([A, P], F32)
            vsel_b = L.tile([A, P * P], BF16)
            m_row = L.tile([1, T], F32R)           # (m-1)*1e9
            ones_r = L.tile([1, P], F32R)
            rm_f = L.tile([P, 1], F32)
            e_full = L.tile([P, T], F32)
            aT_all = L.tile([128, Q * P], F32R)
            scs = [L.tile([A, T // 2], BF16, name=f"sc{j}") for j in range(4)]
            sumh = [L.tile([P, 1], F32, name=f"sumh{j}") for j in range(2)]
            nbnd = L.tile([P, 1], F32)
            sums = L.tile([P, 1], F32)
            rc = L.tile([P, 1], F32)
            rc2 = L.tile([P, 1], F32)

            make_identity(nc, ident)
            nc.vector.tensor_copy(identr[:], ident[:])
            nc.gpsimd.memset(nbnd[:], -float(bound))

            with (
                tc.tile_pool(name="prep", bufs=1) as C,
                tc.tile_pool(name="ps0", bufs=1, space="PSUM") as P0,
                tc.tile_pool(name="pse", bufs=1, space="PSUM") as PE_,
            ):
                # ---- params ---------------------------------------------
                uw_sb = C.tile([A, D], F32)
                wsw_sb = C.tile([A, D], F32)
                sj_sb = C.tile([P, D], F32)
                wsb_sb = C.tile([A, 1], F32)
                v_sb = C.tile([A, 1], F32)
                nc.sync.dma_start(uw_sb[:], uw_dram[:])
                nc.sync.dma_start(wsw_sb[:], wsw_dram[:])
                nc.sync.dma_start(sj_sb[:], sj_dram[:])
                nc.sync.dma_start(wsb_sb[:, 0:1], wsb_dram.ap().rearrange("(a o) -> a o", o=1))
                nc.sync.dma_start(v_sb[:, 0:1], vw_dram.ap().rearrange("o a -> a o"))

                uwT_r = C.tile([128, DC * A], F32R)
                wswT = C.tile([128, DC * A], F32)
                sjT = C.tile([128, DC * P], F32)
                for c in range(DC):
                    tp = P0.tile([128, 128], F32, tag="xtp", bufs=2)
                    nc.tensor.transpose(tp[:], uw_sb[:, c * 128:(c + 1) * 128], ident[:])
                    nc.vector.tensor_copy(uwT_r[:, c * A:(c + 1) * A], tp[:])
                    tp2 = P0.tile([128, 128], F32, tag="xtp", bufs=2)
                    nc.tensor.transpose(tp2[:], wsw_sb[:, c * 128:(c + 1) * 128], ident[:])
                    nc.vector.tensor_copy(wswT[:, c * A:(c + 1) * A], tp2[:])
                    tp3 = P0.tile([128, P], F32, tag="xtp", bufs=2)
                    nc.tensor.transpose(tp3[:], sj_sb[:, c * 128:(c + 1) * 128], ident[0:P, 0:P])
                    nc.vector.tensor_copy(sjT[:, c * P:(c + 1) * P], tp3[:])

                # ws.T = Ws_w @ s_j.T + Ws_b : [A partitions, P]
                ws_ps = P0.tile([A, P], F32, tag="ups", bufs=2)
                for c in range(DC):
                    nc.tensor.matmul(
                        ws_ps[:], wswT[:, c * A:(c + 1) * A], sjT[:, c * P:(c + 1) * P],
                        start=(c == 0), stop=(c == DC - 1),
                    )
                nc.vector.tensor_scalar_add(wsT[:], ws_ps[:], wsb_sb[:, 0:1])

                # selector weights: slice p has column p = v
                vsel_f = C.tile([A, P * P], F32)
                nc.gpsimd.memset(vsel_f[:], 0.0)
                for p in range(P):
                    nc.vector.tensor_copy(vsel_f[:, p * P + p:p * P + p + 1], v_sb[:, 0:1])
                nc.vector.tensor_copy(vsel_b[:], vsel_f[:])

                # mask row: m_row = (m - 1) * 1e9. The int32 mask is staged in
                # the (not yet used) `ex` tile's bytes, then cast+affine in one
                # DVE pass whose f32r output satisfies the fp32r rounding rule.
                nc.sync.dma_start(
                    e_full[0:1, :].bitcast(I32),
                    m_dram.ap().rearrange("q t -> (q t)").rearrange("(o f) -> o f", o=1),
                )
                nc.vector.tensor_scalar(
                    m_row[:], e_full[0:1, :].bitcast(I32), 1.0, 1e9,
                    op0=ALU.subtract, op1=ALU.mult,
                )
                ones_f = C.tile([1, P], F32)
                nc.gpsimd.memset(ones_f[:], 1.0)
                nc.vector.tensor_copy(ones_r[:], ones_f[:])
                rm_i = C.tile([P, 1], I32)
                nc.sync.dma_start(rm_i[:, 0:1], rm_dram.ap().rearrange("(p o) -> p o", o=1))
                nc.vector.tensor_copy(rm_f[:, 0:1], rm_i[:, 0:1])

                # ---- per-half: load x, build uhT, tanh+e loop -----------
                with tc.tile_pool(name="xts", bufs=1) as XT:
                    for h in range(2):
                        q0 = h * QH
                        for i in range(q0, q0 + QH):
                            nc.sync.dma_start(
                                x_all[:, i * D:(i + 1) * D], x_dram[i].bitcast(F32R))
                        for g in range(QH // 4):
                            xt_c = []
                            for c in range(DC):
                                tp = P0.tile([128, 512], F32, tag="xtp", bufs=2)
                                for j in range(4):
                                    iq = q0 + g * 4 + j
                                    nc.tensor.transpose(
                                        tp[:, j * 128:(j + 1) * 128].bitcast(F32R),
                                        x_all[:, iq * D + c * 128: iq * D + (c + 1) * 128],
                                        identr[:],
                                    )
                                xs = XT.tile([128, 512], F32R, tag=f"xs{c}")
                                nc.vector.tensor_copy(xs[:], tp[:])
                                xt_c.append(xs)
                            ups = P0.tile([A, 512], F32, tag="ups", bufs=2)
                            for c in range(DC):
                                nc.tensor.matmul(
                                    ups[:], uwT_r[:, c * A:(c + 1) * A], xt_c[c][:],
                                    start=(c == 0), stop=(c == DC - 1),
                                )
                            gi = h * NCK2 + g
                            nc.vector.tensor_copy(uhT[:, gi * 512:(gi + 1) * 512], ups[:])

                        # tanh + selector-matmul accumulation for this half
                        T2h = T // 2
                        e_ps = PE_.tile([P, T2h], F32, tag="eps", bufs=1)
                        for p in range(P):
                            sc = scs[p % 4]
                            nc.scalar.activation(
                                sc[:], uhT[:, h * T2h:(h + 1) * T2h], AF.Tanh,
                                bias=wsT[:, p:p + 1], scale=1.0,
                            )
                            for c in range(NCK2):
                                nc.tensor.matmul(
                                    e_ps[:, c * 512:(c + 1) * 512],
                                    vsel_b[:, p * P:(p + 1) * P],
                                    sc[:, c * 512:(c + 1) * 512],
                                    start=(p == 0), stop=False,
                                )
                        # additive exp_mask: e += 1 * ((m-1)*1e9)
                        for c in range(NCK2):
                            nc.tensor.matmul(
                                e_ps[:, c * 512:(c + 1) * 512],
                                ones_r[:, 0:P],
                                m_row[:, h * T2h + c * 512: h * T2h + (c + 1) * 512],
                                start=False, stop=True,
                            )
                        # exp(e - bound) evacuates PSUM and accumulates the
                        # half-denominator in one ScalarE pass
                        nc.scalar.activation(
                            e_full[:, h * T2h:(h + 1) * T2h], e_ps[:], AF.Exp,
                            bias=nbnd[:, 0:1], scale=1.0,
                            accum_out=sumh[h][:, 0:1],
                        )

            # ---- softmax normalization scalar (applied on output copies) ---
            nc.vector.tensor_tensor(sums[:, 0:1], sumh[0][:, 0:1], sumh[1][:, 0:1], op=ALU.add)
            nc.vector.reciprocal(rc[:, 0:1], sums[:, 0:1])
            nc.vector.tensor_tensor(rc2[:, 0:1], rc[:, 0:1], rm_f[:, 0:1], op=ALU.mult)

            # ---- apply: out[i] = a[:, i-block] @ x[i] -------------------
            with (
                tc.tile_pool(name="apl", bufs=3) as AP_,
                tc.tile_pool(name="psa", bufs=2, space="PSUM") as PA,
            ):
                for i in range(q):
                    atp = PA.tile([128, P], F32, tag="atp", bufs=4)
                    nc.tensor.transpose(
                        atp[:], e_full[:, i * LE:(i + 1) * LE], ident[0:P, 0:P])
                    nc.vector.tensor_copy(aT_all[:, i * P:(i + 1) * P], atp[:])
                    ops = PA.tile([P, D], F32, tag="ops", bufs=4)
                    nc.tensor.matmul(ops[:], aT_all[:, i * P:(i + 1) * P],
                                     x_all[:, i * D:(i + 1) * D],
                                     start=True, stop=True)
                    osb = AP_.tile([P, D], F32, tag="osb", bufs=6)
                    if i % 3 == 0:
                        nc.vector.tensor_scalar_mul(osb[:], ops[:], rc2[:, 0:1])
                    else:
                        nc.scalar.activation(osb[:], ops[:], AF.Copy,
                                             bias=0.0, scale=rc2[:, 0:1])
                    nc.sync.dma_start(out_dram[i], osb[:])

    nc.compile()
    return nc


_NC_CACHE = {}


def _get_nc(q=Q, bound=12.0):
    key = (q, round(float(bound), 6))
    if key not in _NC_CACHE:
        _NC_CACHE[key] = build_kernel(q, bound)
    return _NC_CACHE[key]


def kernel(exp_tokens, exp_mask, s_j, req_mask, Ws_w, Ws_b, U_w, v_w):
    """Full-input entry point: shard over B across 8 cores, gather output."""
    from concourse.bass_utils import run_bass_kernel_spmd

    bound = float(np.abs(np.asarray(v_w, dtype=np.float64)).sum()) + 1.0
    nc = _get_nc(Q, bound)
    in_maps = []
    for b in range(N_CORES):
        in_maps.append({
            "exp_tokens": np.ascontiguousarray(exp_tokens[b], dtype=np.float32),
            "exp_mask": np.ascontiguousarray(exp_mask[b], dtype=np.int32),
            "s_j": np.ascontiguousarray(s_j[b], dtype=np.float32),
            "req_mask": np.ascontiguousarray(req_mask[b], dtype=np.int32),
            "Ws_w": np.ascontiguousarray(Ws_w, dtype=np.float32),
            "Ws_b": np.ascontiguousarray(Ws_b, dtype=np.float32),
            "U_w": np.ascontiguousarray(U_w, dtype=np.float32),
            "v_w": np.ascontiguousarray(v_w, dtype=np.float32),
        })
    res = run_bass_kernel_spmd(nc, in_maps, core_ids=list(range(N_CORES)))
    out = np.stack([res.results[b]["out"] for b in range(N_CORES)], axis=0)
    return out.astype(np.float32)


def reference_1b(x, m, sj, rm, Ws_w, Ws_b, U_w, v_w):
    """Numpy reference for ONE batch, mirroring the kernel's math (fp64)."""
    q = x.shape[0]
    T = q * LE
    xf = x.reshape(T, D).astype(np.float64)
    ws = sj.astype(np.float64) @ Ws_w.T.astype(np.float64) + Ws_b.astype(np.float64)
    uh = xf @ U_w.T.astype(np.float64)                       # [T, A]
    mf = m.reshape(T).astype(np.float64)
    e = np.tanh(uh[None, :, :] + ws[:, None, :]) @ v_w[0].astype(np.float64)  # [P, T]
    em = e + (mf[None, :] - 1.0) * 1e9
    bnd = np.abs(v_w).sum() + 1.0
    exm = np.exp(em - bnd)
    a = exm / exm.sum(axis=1, keepdims=True) * rm.astype(np.float64)[:, None]
    out = np.zeros((q, P, D))
    for i in range(q):
        out[i] = a[:, i * LE:(i + 1) * LE] @ xf[i * LE:(i + 1) * LE]
    return out


# revision 41
# speedup vs baseline: 1.0101x; 1.0101x over previous
"""Trainium2 Bass kernel for nn_AbilityGammaAttention.

Reference computation (per batch b):
    ws = s_j @ Ws_w.T + Ws_b                      # (P, A)
    uh = exp_tokens @ U_w.T                       # (Q, LE, A)
    e[q,p,t] = v . tanh(uh[q,t,:] + ws[p,:])      # (Q, P, LE)
    e masked by exp_mask (tokens), joint softmax over (Q, LE) per (b, p)
    out[q,p,:] = sum_t a[q,p,t] * exp_tokens[q,t,:], zeroed where req_mask[p]==0

Sharding: data-parallel over B across the 8 NeuronCores (batch b -> core b).

Per-core design:
  - uh is computed transposed (uhT: [A=128 partitions, tokens]) so the per-p
    "+ ws[p,:]" is a per-partition bias fused into the ScalarE tanh (the
    ScalarE tanh over P x tokens x A elements is the bottleneck engine).
  - e for all 32 p is accumulated directly into a PSUM tile [P, tokens-half]
    using a selector weight matrix (column p of slice p = v, rest zeros), so
    per-p PSUM evacuation is avoided; exp_mask lands as an extra accumulated
    (m-1)*1e9 rank-1 matmul, exactly reproducing the reference -1e9 masking.
  - Tokens are processed in two halves with the x-load/transpose/uh prep of
    half 1 overlapping the tanh loop of half 0.
  - Matmuls run in float32r (TF32-like); the tanh->e path runs in bf16
    (tanh output is in [-1,1]).
  - Softmax uses a data-independent shift (bound = sum|v_w|, computed on the
    host at first call) instead of a row max, so exp can never overflow and
    both reduce_max passes disappear; denominators come free via the ScalarE
    Exp accum_out, and the normalization (and req_mask) is folded into the
    per-partition scale of the PSUM->SBUF output copies.
"""

import sys

if "/opt/trn_rl_repo" not in sys.path:
    sys.path.insert(0, "/opt/trn_rl_repo")

import numpy as np

import concourse.bacc as bacc
import concourse.mybir as mybir
from concourse.masks import make_identity
from concourse.tile import TileContext

F32 = mybir.dt.float32
F32R = mybir.dt.float32r
BF16 = mybir.dt.bfloat16
I32 = mybir.dt.int32
AF = mybir.ActivationFunctionType
ALU = mybir.AluOpType

B, Q, LE, D, P, A = 8, 32, 128, 512, 32, 128
N_CORES = 8


def build_kernel(q=Q, bound=12.0, le=LE):
    """Build the per-core kernel (one batch per core). q must be a multiple of 8.

    `bound` is any value >= max possible |e| = sum(|v_w|); exp is shifted by it
    instead of a computed row max (softmax is shift-invariant, and exp(e-bound)
    can never overflow)."""
    T = q * le          # tokens per batch
    GW = 4 * le         # uh-group width (4 q per group)
    DC = D // 128       # contraction chunks (4)
    T2 = T // 2         # tokens per half
    NCK2 = T2 // 512    # 512-column (one PSUM bank) chunks per half
    QH = q // 2         # q per half
    assert NCK2 * 512 == T2 and QH % 4 == 0

    nc = bacc.Bacc("TRN2", target_bir_lowering=False, debug=False)

    x_dram = nc.dram_tensor("exp_tokens", [q, le, D], F32, kind="ExternalInput")
    mr_dram = nc.dram_tensor("m_row_in", [1, T], F32, kind="ExternalInput")
    sj_dram = nc.dram_tensor("s_j", [P, D], F32, kind="ExternalInput")
    rm_dram = nc.dram_tensor("req_mask", [P], I32, kind="ExternalInput")
    wsw_dram = nc.dram_tensor("Ws_w", [A, D], F32, kind="ExternalInput")
    wsb_dram = nc.dram_tensor("Ws_b", [A], F32, kind="ExternalInput")
    uw_dram = nc.dram_tensor("U_w", [A, D], F32, kind="ExternalInput")
    vw_dram = nc.dram_tensor("v_w", [1, A], F32, kind="ExternalInput")
    out_dram = nc.dram_tensor("out", [q, P, D], F32, kind="ExternalOutput")

    with TileContext(nc) as tc:
        with tc.tile_pool(name="live", bufs=1) as L:
            # ---- whole-kernel tensors (base of the SBUF stack) ----------
            ident = L.tile([128, 128], F32)
            identr = L.tile([128, 128], F32R)
            x_all = L.tile([128, q * D], F32R)     # x[i] natural at cols i*D
            uhT = L.tile([A, T], F32)              # U_w @ x.T
            wsT = L.tile([A, P], F32)
            vsel_b = L.tile([A, P * P], BF16)
            m_row = L.tile([1, T], F32R)           # (m-1)*1e9
            ones_r = L.tile([1, P], F32R)
            rm_f = L.tile([P, 1], F32)
            e_full = L.tile([P, T], F32)
            aT_all = L.tile([128, Q * P], F32R)
            scs = [L.tile([A, T // 2], BF16, name=f"sc{j}") for j in range(4)]
            sumh = [L.tile([P, 1], F32, name=f"sumh{j}") for j in range(2)]
            nbnd = L.tile([P, 1], F32)
            sums = L.tile([P, 1], F32)
            rc = L.tile([P, 1], F32)
            rc2 = L.tile([P, 1], F32)

            make_identity(nc, ident)
            nc.vector.tensor_copy(identr[:], ident[:])
            nc.gpsimd.memset(nbnd[:], -float(bound))

            with (
                tc.tile_pool(name="prep", bufs=1) as C,
                tc.tile_pool(name="ps0", bufs=1, space="PSUM") as P0,
                tc.tile_pool(name="pse", bufs=1, space="PSUM") as PE_,
            ):
                # ---- params ---------------------------------------------
                uw_sb = C.tile([A, D], F32)
                wsw_sb = C.tile([A, D], F32)
                sj_sb = C.tile([P, D], F32)
                wsb_sb = C.tile([A, 1], F32)
                v_sb = C.tile([A, 1], F32)
                nc.sync.dma_start(uw_sb[:], uw_dram[:])
                nc.sync.dma_start(wsw_sb[:], wsw_dram[:])
                nc.sync.dma_start(sj_sb[:], sj_dram[:])
                nc.sync.dma_start(wsb_sb[:, 0:1], wsb_dram.ap().rearrange("(a o) -> a o", o=1))
                nc.sync.dma_start(v_sb[:, 0:1], vw_dram.ap().rearrange("o a -> a o"))

                uwT_r = C.tile([128, DC * A], F32R)
                wswT = C.tile([128, DC * A], F32)
                sjT = C.tile([128, DC * P], F32)
                for c in range(DC):
                    tp = P0.tile([128, 128], F32, tag="xtp", bufs=2)
                    nc.tensor.transpose(tp[:], uw_sb[:, c * 128:(c + 1) * 128], ident[:])
                    nc.vector.tensor_copy(uwT_r[:, c * A:(c + 1) * A], tp[:])
                    tp2 = P0.tile([128, 128], F32, tag="xtp", bufs=2)
                    nc.tensor.transpose(tp2[:], wsw_sb[:, c * 128:(c + 1) * 128], ident[:])
                    nc.vector.tensor_copy(wswT[:, c * A:(c + 1) * A], tp2[:])
                    tp3 = P0.tile([128, P], F32, tag="xtp", bufs=2)
                    nc.tensor.transpose(tp3[:], sj_sb[:, c * 128:(c + 1) * 128], ident[0:P, 0:P])
                    nc.vector.tensor_copy(sjT[:, c * P:(c + 1) * P], tp3[:])

                # ws.T = Ws_w @ s_j.T + Ws_b : [A partitions, P]
                ws_ps = P0.tile([A, P], F32, tag="ups", bufs=2)
                for c in range(DC):
                    nc.tensor.matmul(
                        ws_ps[:], wswT[:, c * A:(c + 1) * A], sjT[:, c * P:(c + 1) * P],
                        start=(c == 0), stop=(c == DC - 1),
                    )
                nc.vector.tensor_scalar_add(wsT[:], ws_ps[:], wsb_sb[:, 0:1])

                # selector weights: slice p has column p = v
                vsel_f = C.tile([A, P * P], F32)
                nc.gpsimd.memset(vsel_f[:], 0.0)
                for p in range(P):
                    nc.vector.tensor_copy(vsel_f[:, p * P + p:p * P + p + 1], v_sb[:, 0:1])
                nc.vector.tensor_copy(vsel_b[:], vsel_f[:])

                # mask row (m-1)*1e9 is precomputed on the host
                nc.sync.dma_start(m_row[:], mr_dram.ap().bitcast(F32R))
                ones_f = C.tile([1, P], F32)
                nc.gpsimd.memset(ones_f[:], 1.0)
                nc.vector.tensor_copy(ones_r[:], ones_f[:])
                rm_i = C.tile([P, 1], I32)
                nc.sync.dma_start(rm_i[:, 0:1], rm_dram.ap().rearrange("(p o) -> p o", o=1))
                nc.vector.tensor_copy(rm_f[:, 0:1], rm_i[:, 0:1])

                # ---- per-half: load x, build uhT, tanh+e loop -----------
                with tc.tile_pool(name="xts", bufs=1) as XT:
                    for h in range(2):
                        q0 = h * QH
                        for i in range(q0, q0 + QH):
                            nc.sync.dma_start(
                                x_all[0:le, i * D:(i + 1) * D], x_dram[i].bitcast(F32R))
                        for g in range(QH // 4):
                            xt_c = []
                            for c in range(DC):
                                tp = P0.tile([128, GW], F32, tag="xtp", bufs=2)
                                for j in range(4):
                                    iq = q0 + g * 4 + j
                                    nc.tensor.transpose(
                                        tp[:, j * le:(j + 1) * le].bitcast(F32R),
                                        x_all[0:le, iq * D + c * 128: iq * D + (c + 1) * 128],
                                        identr[0:le, 0:le],
                                    )
                                xs = XT.tile([128, GW], F32R, tag=f"xs{c}")
                                nc.vector.tensor_copy(xs[:], tp[:])
                                xt_c.append(xs)
                            ups = P0.tile([A, GW], F32, tag="ups", bufs=2)
                            for c in range(DC):
                                nc.tensor.matmul(
                                    ups[:], uwT_r[:, c * A:(c + 1) * A], xt_c[c][:],
                                    start=(c == 0), stop=(c == DC - 1),
                                )
                            gi = (h * (QH // 4) + g)
                            nc.vector.tensor_copy(uhT[:, gi * GW:(gi + 1) * GW], ups[:])

                        # tanh + selector-matmul accumulation for this half
                        T2h = T // 2
                        e_ps = PE_.tile([P, T2h], F32, tag="eps", bufs=1)
                        for p in range(P):
                            sc = scs[p % 4]
                            nc.scalar.activation(
                                sc[:], uhT[:, h * T2h:(h + 1) * T2h], AF.Tanh,
                                bias=wsT[:, p:p + 1], scale=1.0,
                            )
                            for c in range(NCK2):
                                nc.tensor.matmul(
                                    e_ps[:, c * 512:(c + 1) * 512],
                                    vsel_b[:, p * P:(p + 1) * P],
                                    sc[:, c * 512:(c + 1) * 512],
                                    start=(p == 0), stop=False,
                                )
                        # additive exp_mask: e += 1 * ((m-1)*1e9)
                        for c in range(NCK2):
                            nc.tensor.matmul(
                                e_ps[:, c * 512:(c + 1) * 512],
                                ones_r[:, 0:P],
                                m_row[:, h * T2h + c * 512: h * T2h + (c + 1) * 512],
                                start=False, stop=True,
                            )
                        # exp(e - bound) evacuates PSUM and accumulates the
                        # half-denominator in one ScalarE pass
                        nc.scalar.activation(
                            e_full[:, h * T2h:(h + 1) * T2h], e_ps[:], AF.Exp,
                            bias=nbnd[:, 0:1], scale=1.0,
                            accum_out=sumh[h][:, 0:1],
                        )

            # ---- softmax normalization scalar (applied on output copies) ---
            nc.vector.tensor_tensor(sums[:, 0:1], sumh[0][:, 0:1], sumh[1][:, 0:1], op=ALU.add)
            nc.vector.reciprocal(rc[:, 0:1], sums[:, 0:1])
            nc.vector.tensor_tensor(rc2[:, 0:1], rc[:, 0:1], rm_f[:, 0:1], op=ALU.mult)

            # ---- apply: out[i] = a[:, i-block] @ x[i] -------------------
            with (
                tc.tile_pool(name="apl", bufs=3) as AP_,
                tc.tile_pool(name="psa", bufs=2, space="PSUM") as PA,
            ):
                for i in range(q):
                    atp = PA.tile([128, P], F32, tag="atp", bufs=4)
                    nc.tensor.transpose(
                        atp[0:le, :], e_full[:, i * le:(i + 1) * le], ident[0:P, 0:P])
                    nc.vector.tensor_copy(aT_all[0:le, i * P:(i + 1) * P], atp[0:le, :])
                    ops = PA.tile([P, D], F32, tag="ops", bufs=4)
                    nc.tensor.matmul(ops[:], aT_all[0:le, i * P:(i + 1) * P],
                                     x_all[0:le, i * D:(i + 1) * D],
                                     start=True, stop=True)
                    osb = AP_.tile([P, D], F32, tag="osb", bufs=6)
                    if i % 3 == 0:
                        nc.vector.tensor_scalar_mul(osb[:], ops[:], rc2[:, 0:1])
                    else:
                        nc.scalar.activation(osb[:], ops[:], AF.Copy,
                                             bias=0.0, scale=rc2[:, 0:1])
                    nc.sync.dma_start(out_dram[i], osb[:])

    nc.compile()
    return nc


_NC_CACHE = {}


def _get_nc(q=Q, bound=12.0, le=LE):
    key = (q, round(float(bound), 6), le)
    if key not in _NC_CACHE:
        _NC_CACHE[key] = build_kernel(q, bound, le)
    return _NC_CACHE[key]


def _compact(exp_tokens, exp_mask, le):
    """Per-(b,q) host compaction: move each q's unmasked tokens to the front,
    pad to `le` slots (padding slots masked out). Exact: masked tokens never
    contribute to softmax or output. Returns None if any q overflows `le`."""
    b, q, full, d = exp_tokens.shape
    counts = exp_mask.sum(axis=2)
    if counts.max() > le:
        return None
    x_c = np.empty((b, q, le, d), dtype=np.float32)
    m_c = np.zeros((b, q, le), dtype=np.float32)
    for bi in range(b):
        for qi in range(q):
            idx = np.flatnonzero(exp_mask[bi, qi])
            n = len(idx)
            x_c[bi, qi, :n] = exp_tokens[bi, qi, idx]
            if n < le:
                x_c[bi, qi, n:] = 0.0
            m_c[bi, qi, :n] = 1.0
    m_row = ((m_c.reshape(b, 1, q * le) - 1.0) * 1e9).astype(np.float32)
    return x_c, m_row


def kernel(exp_tokens, exp_mask, s_j, req_mask, Ws_w, Ws_b, U_w, v_w):
    """Full-input entry point: shard over B across 8 cores, gather output."""
    from concourse.bass_utils import run_bass_kernel_spmd

    exp_tokens = np.asarray(exp_tokens, dtype=np.float32)
    exp_mask = np.asarray(exp_mask, dtype=np.int32)
    bound = float(np.abs(np.asarray(v_w, dtype=np.float64)).sum()) + 1.0

    le = 96
    packed = _compact(exp_tokens, exp_mask, le)
    if packed is None:
        # improbable overflow (>96 of 128 tokens unmasked somewhere):
        # fall back to the uncompacted kernel
        le = LE
        m_c = ((exp_mask.reshape(B, 1, Q * LE).astype(np.float32) - 1.0) * 1e9)
        packed = (exp_tokens, m_c)
    x_c, m_row = packed

    nc = _get_nc(Q, bound, le)
    in_maps = []
    for b in range(N_CORES):
        in_maps.append({
            "exp_tokens": np.ascontiguousarray(x_c[b], dtype=np.float32),
            "m_row_in": np.ascontiguousarray(m_row[b], dtype=np.float32),
            "s_j": np.ascontiguousarray(s_j[b], dtype=np.float32),
            "req_mask": np.ascontiguousarray(req_mask[b], dtype=np.int32),
            "Ws_w": np.ascontiguousarray(Ws_w, dtype=np.float32),
            "Ws_b": np.ascontiguousarray(Ws_b, dtype=np.float32),
            "U_w": np.ascontiguousarray(U_w, dtype=np.float32),
            "v_w": np.ascontiguousarray(v_w, dtype=np.float32),
        })
    res = run_bass_kernel_spmd(nc, in_maps, core_ids=list(range(N_CORES)))
    out = np.stack([res.results[b]["out"] for b in range(N_CORES)], axis=0)
    return out.astype(np.float32)


def reference_1b(x, m, sj, rm, Ws_w, Ws_b, U_w, v_w):
    """Numpy reference for ONE batch, mirroring the kernel's math (fp64)."""
    q = x.shape[0]
    T = q * LE
    xf = x.reshape(T, D).astype(np.float64)
    ws = sj.astype(np.float64) @ Ws_w.T.astype(np.float64) + Ws_b.astype(np.float64)
    uh = xf @ U_w.T.astype(np.float64)                       # [T, A]
    mf = m.reshape(T).astype(np.float64)
    e = np.tanh(uh[None, :, :] + ws[:, None, :]) @ v_w[0].astype(np.float64)  # [P, T]
    em = e + (mf[None, :] - 1.0) * 1e9
    bnd = np.abs(v_w).sum() + 1.0
    exm = np.exp(em - bnd)
    a = exm / exm.sum(axis=1, keepdims=True) * rm.astype(np.float64)[:, None]
    out = np.zeros((q, P, D))
    for i in range(q):
        out[i] = a[:, i * LE:(i + 1) * LE] @ xf[i * LE:(i + 1) * LE]
    return out


# revision 42
# speedup vs baseline: 1.2221x; 1.2099x over previous
"""Trainium2 Bass kernel for nn_AbilityGammaAttention.

Reference computation (per batch b):
    ws = s_j @ Ws_w.T + Ws_b                      # (P, A)
    uh = exp_tokens @ U_w.T                       # (Q, LE, A)
    e[q,p,t] = v . tanh(uh[q,t,:] + ws[p,:])      # (Q, P, LE)
    e masked by exp_mask (tokens), joint softmax over (Q, LE) per (b, p)
    out[q,p,:] = sum_t a[q,p,t] * exp_tokens[q,t,:], zeroed where req_mask[p]==0

Sharding: data-parallel over B across the 8 NeuronCores (batch b -> core b).

Per-core design:
  - uh is computed transposed (uhT: [A=128 partitions, tokens]) so the per-p
    "+ ws[p,:]" is a per-partition bias fused into the ScalarE tanh (the
    ScalarE tanh over P x tokens x A elements is the bottleneck engine).
  - e for all 32 p is accumulated directly into a PSUM tile [P, tokens-half]
    using a selector weight matrix (column p of slice p = v, rest zeros), so
    per-p PSUM evacuation is avoided; exp_mask lands as an extra accumulated
    (m-1)*1e9 rank-1 matmul, exactly reproducing the reference -1e9 masking.
  - Tokens are processed in two halves with the x-load/transpose/uh prep of
    half 1 overlapping the tanh loop of half 0.
  - Matmuls run in float32r (TF32-like); the tanh->e path runs in bf16
    (tanh output is in [-1,1]).
  - Softmax uses a data-independent shift (bound = sum|v_w|, computed on the
    host at first call) instead of a row max, so exp can never overflow and
    both reduce_max passes disappear; denominators come free via the ScalarE
    Exp accum_out, and the normalization (and req_mask) is folded into the
    per-partition scale of the PSUM->SBUF output copies.
"""

import sys

if "/opt/trn_rl_repo" not in sys.path:
    sys.path.insert(0, "/opt/trn_rl_repo")

import numpy as np

import concourse.bacc as bacc
import concourse.mybir as mybir
from concourse.masks import make_identity
from concourse.tile import TileContext

F32 = mybir.dt.float32
F32R = mybir.dt.float32r
BF16 = mybir.dt.bfloat16
I32 = mybir.dt.int32
AF = mybir.ActivationFunctionType
ALU = mybir.AluOpType

B, Q, LE, D, P, A = 8, 32, 128, 512, 32, 128
N_CORES = 8


def build_kernel(q=Q, bound=12.0, le=LE):
    """Build the per-core kernel (one batch per core). q must be a multiple of 8.

    `bound` is any value >= max possible |e| = sum(|v_w|); exp is shifted by it
    instead of a computed row max (softmax is shift-invariant, and exp(e-bound)
    can never overflow)."""
    T = q * le          # tokens per batch
    GW = 4 * le         # uh-group width (4 q per group)
    DC = D // 128       # contraction chunks (4)
    T2 = T // 2         # tokens per half
    NCK2 = T2 // 512    # 512-column (one PSUM bank) chunks per half
    QH = q // 2         # q per half
    assert NCK2 * 512 == T2 and QH % 4 == 0

    nc = bacc.Bacc("TRN2", target_bir_lowering=False, debug=False)

    x_dram = nc.dram_tensor("exp_tokens", [q, le, D], F32, kind="ExternalInput")
    mr_dram = nc.dram_tensor("m_row_in", [1, T], F32, kind="ExternalInput")
    sj_dram = nc.dram_tensor("s_j", [P, D], F32, kind="ExternalInput")
    rm_dram = nc.dram_tensor("req_mask", [P], I32, kind="ExternalInput")
    wsw_dram = nc.dram_tensor("Ws_w", [A, D], F32, kind="ExternalInput")
    wsb_dram = nc.dram_tensor("Ws_b", [A], F32, kind="ExternalInput")
    uw_dram = nc.dram_tensor("U_w", [A, D], F32, kind="ExternalInput")
    vw_dram = nc.dram_tensor("v_w", [1, A], F32, kind="ExternalInput")
    out_dram = nc.dram_tensor("out", [q, P, D], F32, kind="ExternalOutput")

    with TileContext(nc) as tc:
        with tc.tile_pool(name="live", bufs=1) as L:
            # ---- whole-kernel tensors (base of the SBUF stack) ----------
            ident = L.tile([128, 128], F32)
            identr = L.tile([128, 128], F32R)
            x_all = L.tile([128, q * D], F32R)     # x[i] natural at cols i*D
            uhT = L.tile([A, T], F32)              # U_w @ x.T
            wsT = L.tile([A, P], F32)
            vsel_b = L.tile([A, P * P], BF16)
            m_row = L.tile([1, T], F32R)           # (m-1)*1e9
            ones_r = L.tile([1, P], F32R)
            rm_f = L.tile([P, 1], F32)
            e_full = L.tile([P, T], F32)
            aT_all = L.tile([128, Q * P], F32R)
            scs = [L.tile([A, T // 2], BF16, name=f"sc{j}") for j in range(4)]
            sumh = [L.tile([P, 1], F32, name=f"sumh{j}") for j in range(2)]
            nbnd = L.tile([P, 1], F32)
            sums = L.tile([P, 1], F32)
            rc = L.tile([P, 1], F32)
            rc2 = L.tile([P, 1], F32)

            make_identity(nc, ident)
            nc.vector.tensor_copy(identr[:], ident[:])
            nc.gpsimd.memset(nbnd[:], -float(bound))

            with (
                tc.tile_pool(name="prep", bufs=1) as C,
                tc.tile_pool(name="ps0", bufs=1, space="PSUM") as P0,
                tc.tile_pool(name="pse", bufs=1, space="PSUM") as PE_,
            ):
                # ---- params ---------------------------------------------
                uw_sb = C.tile([A, D], F32)
                wsw_sb = C.tile([A, D], F32)
                sj_sb = C.tile([P, D], F32)
                wsb_sb = C.tile([A, 1], F32)
                v_sb = C.tile([A, 1], F32)
                nc.sync.dma_start(uw_sb[:], uw_dram[:])
                nc.sync.dma_start(wsw_sb[:], wsw_dram[:])
                nc.sync.dma_start(sj_sb[:], sj_dram[:])
                nc.sync.dma_start(wsb_sb[:, 0:1], wsb_dram.ap().rearrange("(a o) -> a o", o=1))
                nc.sync.dma_start(v_sb[:, 0:1], vw_dram.ap().rearrange("o a -> a o"))

                uwT_r = C.tile([128, DC * A], F32R)
                wswT = C.tile([128, DC * A], F32)
                sjT = C.tile([128, DC * P], F32)
                for c in range(DC):
                    tp = P0.tile([128, 128], F32, tag="xtp", bufs=2)
                    nc.tensor.transpose(tp[:], uw_sb[:, c * 128:(c + 1) * 128], ident[:])
                    nc.vector.tensor_copy(uwT_r[:, c * A:(c + 1) * A], tp[:])
                    tp2 = P0.tile([128, 128], F32, tag="xtp", bufs=2)
                    nc.tensor.transpose(tp2[:], wsw_sb[:, c * 128:(c + 1) * 128], ident[:])
                    nc.vector.tensor_copy(wswT[:, c * A:(c + 1) * A], tp2[:])
                    tp3 = P0.tile([128, P], F32, tag="xtp", bufs=2)
                    nc.tensor.transpose(tp3[:], sj_sb[:, c * 128:(c + 1) * 128], ident[0:P, 0:P])
                    nc.vector.tensor_copy(sjT[:, c * P:(c + 1) * P], tp3[:])

                # ws.T = Ws_w @ s_j.T + Ws_b : [A partitions, P]
                ws_ps = P0.tile([A, P], F32, tag="ups", bufs=2)
                for c in range(DC):
                    nc.tensor.matmul(
                        ws_ps[:], wswT[:, c * A:(c + 1) * A], sjT[:, c * P:(c + 1) * P],
                        start=(c == 0), stop=(c == DC - 1),
                    )
                nc.vector.tensor_scalar_add(wsT[:], ws_ps[:], wsb_sb[:, 0:1])

                # selector weights: slice p has column p = v
                vsel_f = C.tile([A, P * P], F32)
                nc.gpsimd.memset(vsel_f[:], 0.0)
                for p in range(P):
                    nc.vector.tensor_copy(vsel_f[:, p * P + p:p * P + p + 1], v_sb[:, 0:1])
                nc.vector.tensor_copy(vsel_b[:], vsel_f[:])

                # mask row (m-1)*1e9 is precomputed on the host
                nc.sync.dma_start(m_row[:], mr_dram.ap().bitcast(F32R))
                ones_f = C.tile([1, P], F32)
                nc.gpsimd.memset(ones_f[:], 1.0)
                nc.vector.tensor_copy(ones_r[:], ones_f[:])
                rm_i = C.tile([P, 1], I32)
                nc.sync.dma_start(rm_i[:, 0:1], rm_dram.ap().rearrange("(p o) -> p o", o=1))
                nc.vector.tensor_copy(rm_f[:, 0:1], rm_i[:, 0:1])

                # ---- per-half: load x, build uhT, tanh+e loop -----------
                with tc.tile_pool(name="xts", bufs=1) as XT:
                    for h in range(2):
                        q0 = h * QH
                        for i in range(q0, q0 + QH):
                            nc.sync.dma_start(
                                x_all[0:le, i * D:(i + 1) * D], x_dram[i].bitcast(F32R))
                        for g in range(QH // 4):
                            xt_c = []
                            for c in range(DC):
                                tp = P0.tile([128, GW], F32, tag="xtp", bufs=2)
                                for j in range(4):
                                    iq = q0 + g * 4 + j
                                    nc.tensor.transpose(
                                        tp[:, j * le:(j + 1) * le].bitcast(F32R),
                                        x_all[0:le, iq * D + c * 128: iq * D + (c + 1) * 128],
                                        identr[0:le, 0:le],
                                    )
                                xs = XT.tile([128, GW], F32R, tag=f"xs{c}")
                                nc.vector.tensor_copy(xs[:], tp[:])
                                xt_c.append(xs)
                            ups = P0.tile([A, GW], F32, tag="ups", bufs=2)
                            for c in range(DC):
                                nc.tensor.matmul(
                                    ups[:], uwT_r[:, c * A:(c + 1) * A], xt_c[c][:],
                                    start=(c == 0), stop=(c == DC - 1),
                                )
                            gi = (h * (QH // 4) + g)
                            nc.vector.tensor_copy(uhT[:, gi * GW:(gi + 1) * GW], ups[:])

                        # tanh + selector-matmul accumulation for this half
                        T2h = T // 2
                        e_ps = PE_.tile([P, T2h], F32, tag="eps", bufs=1)
                        for p in range(P):
                            sc = scs[p % 4]
                            nc.scalar.activation(
                                sc[:], uhT[:, h * T2h:(h + 1) * T2h], AF.Tanh,
                                bias=wsT[:, p:p + 1], scale=1.0,
                            )
                            for c in range(NCK2):
                                nc.tensor.matmul(
                                    e_ps[:, c * 512:(c + 1) * 512],
                                    vsel_b[:, p * P:(p + 1) * P],
                                    sc[:, c * 512:(c + 1) * 512],
                                    start=(p == 0), stop=False,
                                )
                        # additive exp_mask: e += 1 * ((m-1)*1e9)
                        for c in range(NCK2):
                            nc.tensor.matmul(
                                e_ps[:, c * 512:(c + 1) * 512],
                                ones_r[:, 0:P],
                                m_row[:, h * T2h + c * 512: h * T2h + (c + 1) * 512],
                                start=False, stop=True,
                            )
                        # exp(e - bound) evacuates PSUM and accumulates the
                        # half-denominator in one ScalarE pass
                        nc.scalar.activation(
                            e_full[:, h * T2h:(h + 1) * T2h], e_ps[:], AF.Exp,
                            bias=nbnd[:, 0:1], scale=1.0,
                            accum_out=sumh[h][:, 0:1],
                        )

            # ---- softmax normalization scalar (applied on output copies) ---
            nc.vector.tensor_tensor(sums[:, 0:1], sumh[0][:, 0:1], sumh[1][:, 0:1], op=ALU.add)
            nc.vector.reciprocal(rc[:, 0:1], sums[:, 0:1])
            nc.vector.tensor_tensor(rc2[:, 0:1], rc[:, 0:1], rm_f[:, 0:1], op=ALU.mult)

            # ---- apply: out[i] = a[:, i-block] @ x[i] -------------------
            with (
                tc.tile_pool(name="apl", bufs=3) as AP_,
                tc.tile_pool(name="psa", bufs=2, space="PSUM") as PA,
            ):
                for i in range(q):
                    atp = PA.tile([128, P], F32, tag="atp", bufs=4)
                    nc.tensor.transpose(
                        atp[0:le, :], e_full[:, i * le:(i + 1) * le], ident[0:P, 0:P])
                    nc.vector.tensor_copy(aT_all[0:le, i * P:(i + 1) * P], atp[0:le, :])
                    ops = PA.tile([P, D], F32, tag="ops", bufs=4)
                    nc.tensor.matmul(ops[:], aT_all[0:le, i * P:(i + 1) * P],
                                     x_all[0:le, i * D:(i + 1) * D],
                                     start=True, stop=True)
                    osb = AP_.tile([P, D], F32, tag="osb", bufs=6)
                    if i % 3 == 0:
                        nc.vector.tensor_scalar_mul(osb[:], ops[:], rc2[:, 0:1])
                    else:
                        nc.scalar.activation(osb[:], ops[:], AF.Copy,
                                             bias=0.0, scale=rc2[:, 0:1])
                    nc.sync.dma_start(out_dram[i], osb[:])

    nc.compile()
    return nc


_NC_CACHE = {}
LAST_NC = None


def _get_nc(q=Q, bound=12.0, le=LE):
    key = (q, round(float(bound), 6), le)
    if key not in _NC_CACHE:
        _NC_CACHE[key] = build_kernel(q, bound, le)
    return _NC_CACHE[key]


def _compact(exp_tokens, exp_mask, le):
    """Per-(b,q) host compaction: move each q's unmasked tokens to the front,
    pad to `le` slots (padding slots masked out). Exact: masked tokens never
    contribute to softmax or output. Returns None if any q overflows `le`."""
    b, q, full, d = exp_tokens.shape
    counts = exp_mask.sum(axis=2)
    if counts.max() > le:
        return None
    x_c = np.empty((b, q, le, d), dtype=np.float32)
    m_c = np.zeros((b, q, le), dtype=np.float32)
    for bi in range(b):
        for qi in range(q):
            idx = np.flatnonzero(exp_mask[bi, qi])
            n = len(idx)
            x_c[bi, qi, :n] = exp_tokens[bi, qi, idx]
            if n < le:
                x_c[bi, qi, n:] = 0.0
            m_c[bi, qi, :n] = 1.0
    m_row = ((m_c.reshape(b, 1, q * le) - 1.0) * 1e9).astype(np.float32)
    return x_c, m_row


def kernel(exp_tokens, exp_mask, s_j, req_mask, Ws_w, Ws_b, U_w, v_w):
    """Full-input entry point: shard over B across 8 cores, gather output."""
    from concourse.bass_utils import run_bass_kernel_spmd

    exp_tokens = np.asarray(exp_tokens, dtype=np.float32)
    exp_mask = np.asarray(exp_mask, dtype=np.int32)
    bound = float(np.abs(np.asarray(v_w, dtype=np.float64)).sum()) + 1.0

    le = 96
    packed = _compact(exp_tokens, exp_mask, le)
    if packed is None:
        # improbable overflow (>96 of 128 tokens unmasked somewhere):
        # fall back to the uncompacted kernel
        le = LE
        m_c = ((exp_mask.reshape(B, 1, Q * LE).astype(np.float32) - 1.0) * 1e9)
        packed = (exp_tokens, m_c)
    x_c, m_row = packed

    nc = _get_nc(Q, bound, le)
    global LAST_NC
    LAST_NC = nc
    in_maps = []
    for b in range(N_CORES):
        in_maps.append({
            "exp_tokens": np.ascontiguousarray(x_c[b], dtype=np.float32),
            "m_row_in": np.ascontiguousarray(m_row[b], dtype=np.float32),
            "s_j": np.ascontiguousarray(s_j[b], dtype=np.float32),
            "req_mask": np.ascontiguousarray(req_mask[b], dtype=np.int32),
            "Ws_w": np.ascontiguousarray(Ws_w, dtype=np.float32),
            "Ws_b": np.ascontiguousarray(Ws_b, dtype=np.float32),
            "U_w": np.ascontiguousarray(U_w, dtype=np.float32),
            "v_w": np.ascontiguousarray(v_w, dtype=np.float32),
        })
    res = run_bass_kernel_spmd(nc, in_maps, core_ids=list(range(N_CORES)))
    out = np.stack([res.results[b]["out"] for b in range(N_CORES)], axis=0)
    return out.astype(np.float32)


def reference_1b(x, m, sj, rm, Ws_w, Ws_b, U_w, v_w):
    """Numpy reference for ONE batch, mirroring the kernel's math (fp64)."""
    q = x.shape[0]
    T = q * LE
    xf = x.reshape(T, D).astype(np.float64)
    ws = sj.astype(np.float64) @ Ws_w.T.astype(np.float64) + Ws_b.astype(np.float64)
    uh = xf @ U_w.T.astype(np.float64)                       # [T, A]
    mf = m.reshape(T).astype(np.float64)
    e = np.tanh(uh[None, :, :] + ws[:, None, :]) @ v_w[0].astype(np.float64)  # [P, T]
    em = e + (mf[None, :] - 1.0) * 1e9
    bnd = np.abs(v_w).sum() + 1.0
    exm = np.exp(em - bnd)
    a = exm / exm.sum(axis=1, keepdims=True) * rm.astype(np.float64)[:, None]
    out = np.zeros((q, P, D))
    for i in range(q):
        out[i] = a[:, i * LE:(i + 1) * LE] @ xf[i * LE:(i + 1) * LE]
    return out
